# revision 23
# baseline (speedup 1.0000x reference)
"""Trainium2 Bass kernel for adaptive_high_order_residual_v2 (ORDER=2 masked
sign-binarization, per-row stats).

Full-input contract: kernel(x, mask) takes the complete (4096, 11008) arrays,
shards rows across 8 NeuronCores (512 rows each; per-row reductions make this
embarrassingly parallel), runs one SPMD Bass program, and concatenates the
per-core outputs.

Math per row (ORDER = 2, exact restructuring of the reference):
  t    = x * m                      (masked input)
  mean1 = sum(t)/cnt ; var1 = sum(t^2)/cnt - mean1^2 ; s1 = sqrt(var1 * 2/pi)
  b1   = sign(x - mean1)            (valid entries only; invalid masked later)
  q    = (x - s1*b1 - mean1) * m    (this is residual_2 of the reference)
  mean2 = sum(q)/cnt ; var2 = sum(q^2)/cnt - mean2^2 ; s2 = sqrt(var2 * 2/pi)
  b2   = sign(q - mean2)
  out  = ((mean1 + mean2) + s1*b1 + s2*b2) * m
"""

import os
import sys

import numpy as np

sys.path.insert(0, "/opt/trn_rl_repo")

R = 512          # rows per core
N = 11008        # columns
P = 128          # SBUF partitions per row-block
NBLK = R // P    # 4 blocks per core
CW = 2752        # column chunk width
NCH = N // CW    # 4 chunks per block
NCORES = 8
C2 = 0.6366197723675814  # 2/pi

_CACHE = {}


def _build_program():
    import concourse.bacc as bacc
    import concourse.mybir as mybir
    from concourse.tile import TileContext

    F32 = mybir.dt.float32
    BF16 = mybir.dt.bfloat16
    U8 = mybir.dt.uint8
    Alu = mybir.AluOpType
    Act = mybir.ActivationFunctionType
    AX = mybir.AxisListType.X

    nc = bacc.Bacc()
    x = nc.dram_tensor("x", [R, N], F32, kind="ExternalInput")
    mk = nc.dram_tensor("mask", [R, N], U8, kind="ExternalInput")
    out = nc.dram_tensor("out", [R, N], F32, kind="ExternalOutput")

    with TileContext(nc) as tc:
        with (
            tc.tile_pool(name="xq", bufs=6) as xq_pool,
            tc.tile_pool(name="m8", bufs=6) as m8_pool,
            tc.tile_pool(name="b1p", bufs=5) as b1_pool,
            tc.tile_pool(name="b2p", bufs=2) as b2_pool,
            tc.tile_pool(name="w", bufs=5) as w_pool,
            tc.tile_pool(name="scr", bufs=2) as scr_pool,
            tc.tile_pool(name="sc", bufs=2) as sc_pool,
        ):
            for b in range(NBLK):
                r0 = b * P

                xt = [
                    xq_pool.tile([P, CW], F32, name=f"xt{b}_{c}", tag="xq")
                    for c in range(NCH)
                ]
                mt = [
                    m8_pool.tile([P, CW], U8, name=f"mt{b}_{c}", tag="m8")
                    for c in range(NCH)
                ]
                b1t = [
                    b1_pool.tile([P, CW], BF16, name=f"b1_{b}_{c}", tag="b1")
                    for c in range(NCH)
                ]
                # accumulators, chunk-major interleave: col = c*nq + q so the
                # pairwise tree reduce uses contiguous 2-D slices
                acc1 = sc_pool.tile([P, 3 * NCH], F32, name=f"acc1_{b}", tag="acc1")
                acc2 = sc_pool.tile([P, 2 * NCH], F32, name=f"acc2_{b}", tag="acc2")
                st1 = sc_pool.tile([P, 3], F32, name=f"st1_{b}", tag="st1")
                st2 = sc_pool.tile([P, 2], F32, name=f"st2_{b}", tag="st2")
                # row-scalar vector: cntc, inv, mean1, nm1, e1, nv1, v1c, s1,
                # ns1, mean2, nm2, e2, nv2, v2c, s2, K, tmp...
                sv = sc_pool.tile([P, 24], F32, name=f"sv_{b}", tag="sv")

                def col(t, i):
                    return t[:, i : i + 1]

                # ---------------- stage 1: masked first-order stats ---------
                for c in range(NCH):
                    nc.sync.dma_start(xt[c][:], x[r0 : r0 + P, c * CW : (c + 1) * CW])
                    nc.sync.dma_start(mt[c][:], mk[r0 : r0 + P, c * CW : (c + 1) * CW])
                    scr_a = scr_pool.tile(
                        [P, CW], BF16, name=f"scra{b}_{c}", tag="scr"
                    )
                    # cnt partial: copy-cast mask, accumulate row-sum.
                    # (accum-variant ACT ISA struct also has one wait slot —
                    # absorb waits into a plain tiny scalar copy first.)
                    nc.scalar.activation(
                        scr_a[:], mt[c][:], Act.Copy, accum_out=col(acc1, c * 3 + 0)
                    )
                    # T = x*m with accum -> r1 partial
                    tt = w_pool.tile([P, CW], F32, name=f"tt{b}_{c}", tag="w")
                    nc.vector.scalar_tensor_tensor(
                        tt[:],
                        xt[c][:],
                        1.0,
                        mt[c][:],
                        Alu.bypass,
                        Alu.mult,
                        accum_out=col(acc1, c * 3 + 1),
                    )
                    # r2 partial: sum(T^2) via ACT Square accumulate
                    scr_b = scr_pool.tile(
                        [P, CW], BF16, name=f"scrb{b}_{c}", tag="scr"
                    )
                    nc.scalar.activation(
                        scr_b[:], tt[:], Act.Square, accum_out=col(acc1, c * 3 + 2)
                    )

                # reduce chunk partials pairwise ([P, q*NCH] -> [P, q]) with
                # plain TT adds: the TensorReduce ISA struct has one wait
                # slot, TT has two.
                def pair_reduce(dst, acc, nq, red):
                    # chunk-major cols: halves are contiguous 2-D slices
                    h = 2 * nq
                    nc.vector.tensor_add(red[:, 0:h], acc[:, 0:h], acc[:, h : 2 * h])
                    nc.vector.tensor_add(dst, red[:, 0:nq], red[:, nq:h])

                red1 = sc_pool.tile([P, 6], F32, name=f"red1_{b}", tag="red1")
                pair_reduce(st1[:], acc1[:], 3, red1)
                cnt, r1, r2 = col(st1, 0), col(st1, 1), col(st1, 2)
                cntc, inv = col(sv, 0), col(sv, 1)
                mean1, nm1, e1 = col(sv, 2), col(sv, 3), col(sv, 4)
                nv1, v1c, s1, ns1 = col(sv, 5), col(sv, 6), col(sv, 7), col(sv, 8)
                tA, tB, tC, tD = col(sv, 16), col(sv, 17), col(sv, 18), col(sv, 19)
                tE, tF = col(sv, 20), col(sv, 21)

                def newton_sqrt(dst, seed, vsq, t1, t2, mid, vh):
                    # dst = sqrt(vsq) refined from seed (2 Newton steps),
                    # y' = 0.5*y + 0.5*vsq/y. TT/TS only (the STT ISA struct
                    # allows a single sync wait). No aliasing between temps.
                    nc.vector.tensor_scalar(vh[:], vsq[:], 0.5, None, Alu.mult)
                    cur = seed
                    for it in range(2):
                        nc.vector.reciprocal(t1[:], cur[:])
                        nc.vector.tensor_mul(t2[:], vh[:], t1[:])
                        nc.vector.tensor_scalar(t1[:], cur[:], 0.5, None, Alu.mult)
                        nxt = dst if it == 1 else mid
                        nc.vector.tensor_add(nxt[:], t1[:], t2[:])
                        cur = nxt

                nc.vector.tensor_scalar(cntc, cnt, 1.0, None, Alu.max)
                nc.vector.reciprocal(inv, cntc)
                nc.vector.tensor_mul(mean1, r1, inv)
                nc.vector.tensor_scalar(nm1, mean1, -1.0, None, Alu.mult)
                nc.vector.tensor_mul(e1, r2, inv)
                # nv1 = mean1^2, then var1 = e1 - mean1^2 (TT/TS only)
                nc.vector.tensor_mul(nv1, mean1, mean1)
                nc.vector.tensor_sub(tE, e1, nv1)
                # v1c = max(C2*var1, tiny) = var1*(2/pi), clamped
                nc.vector.tensor_scalar(v1c, tE, C2, 1e-30, Alu.mult, Alu.max)
                nc.scalar.activation(tC, v1c, Act.Sqrt)
                newton_sqrt(s1, tC, v1c, tA, tB, tD, tF)
                nc.vector.tensor_scalar(ns1, s1, -1.0, None, Alu.mult)

                # ---------------- stage 2: residual q + second-order stats --
                for c in range(NCH):
                    nc.scalar.activation(b1t[c][:], xt[c][:], Act.Sign, bias=nm1)
                    q0 = w_pool.tile([P, CW], F32, name=f"q0_{b}_{c}", tag="w")
                    # q0 = x - s1*b1
                    nc.vector.scalar_tensor_tensor(
                        q0[:], b1t[c][:], ns1, xt[c][:], Alu.mult, Alu.add
                    )
                    # Q = (q0 - mean1)*m, written over the X tile; accum -> sum(q)
                    nc.vector.scalar_tensor_tensor(
                        xt[c][:],
                        q0[:],
                        mean1,
                        mt[c][:],
                        Alu.subtract,
                        Alu.mult,
                        accum_out=col(acc2, c * 2 + 0),
                    )
                    scr_c = scr_pool.tile(
                        [P, CW], BF16, name=f"scrc{b}_{c}", tag="scr"
                    )
                    nc.scalar.activation(
                        scr_c[:], xt[c][:], Act.Square, accum_out=col(acc2, c * 2 + 1)
                    )

                red2 = sc_pool.tile([P, 4], F32, name=f"red2_{b}", tag="red2")
                pair_reduce(st2[:], acc2[:], 2, red2)
                sq, sq2 = col(st2, 0), col(st2, 1)
                mean2, nm2, e2 = col(sv, 9), col(sv, 10), col(sv, 11)
                nv2, v2c, s2, kk = col(sv, 12), col(sv, 13), col(sv, 14), col(sv, 15)

                nc.vector.tensor_mul(mean2, sq, inv)
                nc.vector.tensor_scalar(nm2, mean2, -1.0, None, Alu.mult)
                nc.vector.tensor_mul(e2, sq2, inv)
                nc.vector.tensor_mul(nv2, mean2, mean2)
                nc.vector.tensor_sub(tE, e2, nv2)
                nc.vector.tensor_scalar(v2c, tE, C2, 1e-30, Alu.mult, Alu.max)
                nc.scalar.activation(tC, v2c, Act.Sqrt)
                newton_sqrt(s2, tC, v2c, tA, tB, tD, tF)
                nc.vector.tensor_add(kk, mean1, mean2)

                # ---------------- stage 3: output assembly ------------------
                for c in range(NCH):
                    b2t = b2_pool.tile([P, CW], BF16, name=f"b2_{b}_{c}", tag="b2")
                    nc.scalar.activation(b2t[:], xt[c][:], Act.Sign, bias=nm2)
                    p1 = w_pool.tile([P, CW], F32, name=f"p1_{b}_{c}", tag="w")
                    # p1 = s1*b1 + K
                    nc.vector.tensor_scalar(p1[:], b1t[c][:], s1, kk, Alu.mult, Alu.add)
                    p2 = w_pool.tile([P, CW], F32, name=f"p2_{b}_{c}", tag="w")
                    # p2 = s2*b2 + p1
                    nc.vector.scalar_tensor_tensor(
                        p2[:], b2t[:], s2, p1[:], Alu.mult, Alu.add
                    )
                    oc = w_pool.tile([P, CW], F32, name=f"oc_{b}_{c}", tag="w")
                    nc.vector.tensor_mul(oc[:], p2[:], mt[c][:])
                    nc.sync.dma_start(
                        out[r0 : r0 + P, c * CW : (c + 1) * CW], oc[:]
                    )

    return nc


def get_program():
    if "nc" not in _CACHE:
        nc = _build_program()
        # Bacc defers register allocation etc. to compile()/finalize();
        # the spmd exec path serializes without finalizing.
        nc.finalize()
        _CACHE["nc"] = nc
    return _CACHE["nc"]


def kernel(x: np.ndarray, mask: np.ndarray) -> np.ndarray:
    from concourse.bass_utils import run_bass_kernel_spmd

    x = np.ascontiguousarray(np.asarray(x, dtype=np.float32))
    mask_u8 = np.ascontiguousarray(np.asarray(mask)).view(np.uint8)
    assert x.shape == (R * NCORES, N), x.shape
    assert mask_u8.shape == (R * NCORES, N), mask_u8.shape

    nc = get_program()
    in_maps = [
        {
            "x": x[k * R : (k + 1) * R],
            "mask": mask_u8[k * R : (k + 1) * R],
        }
        for k in range(NCORES)
    ]
    res = run_bass_kernel_spmd(nc, in_maps, core_ids=list(range(NCORES)))
    return np.concatenate([r["out"] for r in res.results], axis=0)


if __name__ == "__main__":
    xs = np.random.randn(R * NCORES, N).astype(np.float32)
    ms = (np.random.randint(0, 2, (R * NCORES, N))).astype(bool)
    y = kernel(xs, ms)
    print(y.shape, y.dtype)


# revision 25
# speedup vs baseline: 1.0318x; 1.0318x over previous
"""Trainium2 Bass kernel for adaptive_high_order_residual_v2 (ORDER=2 masked
sign-binarization, per-row stats).

Full-input contract: kernel(x, mask) takes the complete (4096, 11008) arrays,
shards rows across 8 NeuronCores (512 rows each; per-row reductions make this
embarrassingly parallel), runs one SPMD Bass program, and concatenates the
per-core outputs.

Math per row (ORDER = 2, exact restructuring of the reference):
  t    = x * m                      (masked input)
  mean1 = sum(t)/cnt ; var1 = sum(t^2)/cnt - mean1^2 ; s1 = sqrt(var1 * 2/pi)
  b1   = sign(x - mean1)            (valid entries only; invalid masked later)
  q    = (|x - mean1| - s1) * b1*m  (== residual_2 of the reference: d - s1*b1)
  mean2 = sum(q)/cnt ; var2 = sum(q^2)/cnt - mean2^2 ; s2 = sqrt(var2 * 2/pi)
  b2   = sign(q - mean2)
  out  = ((mean1 + mean2) + s1*b1 + s2*b2) * m

Engine split per 128x2752 chunk:
  ACT: mask cast (+cnt accum), square(T) (+r2), Sign->b1, Abs->|d|,
       square(q) (+sum q^2), Sign->b2
  DVE: T=x*m (+r1 accum), b1m=b1*m16 (bf16 2x), q=(|d|-s1)*b1m (+sum q),
       p1=s1*b1m+K (2x), p2=s2*b2+p1, out=p2*m
"""

import sys

import numpy as np

sys.path.insert(0, "/opt/trn_rl_repo")

R = 512          # rows per core
N = 11008        # columns
P = 128          # SBUF partitions per row-block
NBLK = R // P    # 4 blocks per core
CW = 2752        # column chunk width
NCH = N // CW    # 4 chunks per block
NCORES = 8
C2 = 0.6366197723675814  # 2/pi

_CACHE = {}


def _build_program():
    import concourse.bacc as bacc
    import concourse.mybir as mybir
    from concourse.tile import TileContext

    F32 = mybir.dt.float32
    BF16 = mybir.dt.bfloat16
    U8 = mybir.dt.uint8
    Alu = mybir.AluOpType
    Act = mybir.ActivationFunctionType

    nc = bacc.Bacc()
    x = nc.dram_tensor("x", [R, N], F32, kind="ExternalInput")
    mk = nc.dram_tensor("mask", [R, N], U8, kind="ExternalInput")
    out = nc.dram_tensor("out", [R, N], F32, kind="ExternalOutput")

    with TileContext(nc) as tc:
        with (
            tc.tile_pool(name="xq", bufs=6) as xq_pool,    # x tile, later holds q
            tc.tile_pool(name="m8", bufs=3) as m8_pool,    # raw u8 mask (cast only)
            tc.tile_pool(name="m16", bufs=5) as m16_pool,  # bf16 mask
            tc.tile_pool(name="b1p", bufs=3) as b1_pool,   # unmasked sign1
            tc.tile_pool(name="bmp", bufs=5) as bm_pool,   # masked sign1 (bf16)
            tc.tile_pool(name="b2p", bufs=3) as b2_pool,   # sign2 + ACT garbage
            tc.tile_pool(name="w", bufs=4) as w_pool,      # f32 rotating work
            tc.tile_pool(name="sc", bufs=2) as sc_pool,    # scalars + accums
        ):
            for b in range(NBLK):
                r0 = b * P

                xt = [
                    xq_pool.tile([P, CW], F32, name=f"xt{b}_{c}", tag="xq")
                    for c in range(NCH)
                ]
                mt = [
                    m8_pool.tile([P, CW], U8, name=f"mt{b}_{c}", tag="m8")
                    for c in range(NCH)
                ]
                m16 = [
                    m16_pool.tile([P, CW], BF16, name=f"m16_{b}_{c}", tag="m16")
                    for c in range(NCH)
                ]
                bm = [
                    bm_pool.tile([P, CW], BF16, name=f"bm{b}_{c}", tag="bm")
                    for c in range(NCH)
                ]
                # accumulators, chunk-major interleave: col = c*nq + q so the
                # pairwise tree reduce uses contiguous 2-D slices
                acc1 = sc_pool.tile([P, 3 * NCH], F32, name=f"acc1_{b}", tag="acc1")
                acc2 = sc_pool.tile([P, 2 * NCH], F32, name=f"acc2_{b}", tag="acc2")
                st1 = sc_pool.tile([P, 3], F32, name=f"st1_{b}", tag="st1")
                st2 = sc_pool.tile([P, 2], F32, name=f"st2_{b}", tag="st2")
                red1 = sc_pool.tile([P, 6], F32, name=f"red1_{b}", tag="red1")
                red2 = sc_pool.tile([P, 4], F32, name=f"red2_{b}", tag="red2")
                sv = sc_pool.tile([P, 24], F32, name=f"sv_{b}", tag="sv")

                def col(t, i):
                    return t[:, i : i + 1]

                # ------------- stage 1: masked first-order stats -------------
                for c in range(NCH):
                    nc.sync.dma_start(xt[c][:], x[r0 : r0 + P, c * CW : (c + 1) * CW])
                    nc.sync.dma_start(mt[c][:], mk[r0 : r0 + P, c * CW : (c + 1) * CW])
                    # mask cast to bf16 + cnt partial
                    nc.scalar.activation(
                        m16[c][:], mt[c][:], Act.Copy, accum_out=col(acc1, c * 3 + 0)
                    )
                    # T = x*m + r1 partial
                    tt = w_pool.tile([P, CW], F32, name=f"tt{b}_{c}", tag="w")
                    nc.vector.scalar_tensor_tensor(
                        tt[:],
                        xt[c][:],
                        1.0,
                        mt[c][:],
                        Alu.bypass,
                        Alu.mult,
                        accum_out=col(acc1, c * 3 + 1),
                    )
                    # r2 partial: sum(T^2); output value unused
                    g1 = b2_pool.tile([P, CW], BF16, name=f"g1_{b}_{c}", tag="b2")
                    nc.scalar.activation(
                        g1[:], tt[:], Act.Square, accum_out=col(acc1, c * 3 + 2)
                    )

                # pairwise chunk reduce ([P, nq*NCH] -> [P, nq]); plain TT adds
                def pair_reduce(dst, acc, nq, red):
                    h = 2 * nq
                    nc.vector.tensor_add(red[:, 0:h], acc[:, 0:h], acc[:, h : 2 * h])
                    nc.vector.tensor_add(dst, red[:, 0:nq], red[:, nq:h])

                pair_reduce(st1[:], acc1[:], 3, red1)
                cnt, r1, r2 = col(st1, 0), col(st1, 1), col(st1, 2)
                cntc, inv = col(sv, 0), col(sv, 1)
                mean1, nm1, e1 = col(sv, 2), col(sv, 3), col(sv, 4)
                nv1, v1c, s1 = col(sv, 5), col(sv, 6), col(sv, 7)
                tA, tB, tC, tD = col(sv, 16), col(sv, 17), col(sv, 18), col(sv, 19)
                tE, tF = col(sv, 20), col(sv, 21)

                def newton_sqrt(dst, seed, vsq, t1, t2, mid, vh):
                    # dst = sqrt(vsq), two Newton steps from the ACT seed.
                    # TT/TS only (the STT ISA struct allows one sync wait).
                    nc.vector.tensor_scalar(vh[:], vsq[:], 0.5, None, Alu.mult)
                    cur = seed
                    for it in range(2):
                        nc.vector.reciprocal(t1[:], cur[:])
                        nc.vector.tensor_mul(t2[:], vh[:], t1[:])
                        nc.vector.tensor_scalar(t1[:], cur[:], 0.5, None, Alu.mult)
                        nxt = dst if it == 1 else mid
                        nc.vector.tensor_add(nxt[:], t1[:], t2[:])
                        cur = nxt

                nc.vector.tensor_scalar(cntc, cnt, 1.0, None, Alu.max)
                nc.vector.reciprocal(inv, cntc)
                nc.vector.tensor_mul(mean1, r1, inv)
                nc.vector.tensor_scalar(nm1, mean1, -1.0, None, Alu.mult)
                nc.vector.tensor_mul(e1, r2, inv)
                nc.vector.tensor_mul(nv1, mean1, mean1)
                nc.vector.tensor_sub(tE, e1, nv1)
                nc.vector.tensor_scalar(v1c, tE, C2, 1e-30, Alu.mult, Alu.max)
                nc.scalar.activation(tC, v1c, Act.Sqrt)
                newton_sqrt(s1, tC, v1c, tA, tB, tD, tF)

                # ------------- stage 2: residual q + second-order stats ------
                for c in range(NCH):
                    b1t = b1_pool.tile([P, CW], BF16, name=f"b1_{b}_{c}", tag="b1")
                    nc.scalar.activation(b1t[:], xt[c][:], Act.Sign, bias=nm1)
                    ab = w_pool.tile([P, CW], F32, name=f"ab{b}_{c}", tag="w")
                    nc.scalar.activation(ab[:], xt[c][:], Act.Abs, bias=nm1)
                    # masked sign1 (bf16 2x)
                    nc.vector.tensor_mul(bm[c][:], b1t[:], m16[c][:])
                    # q = (|d| - s1) * b1m, overwrites the x tile; accum sum(q)
                    nc.vector.scalar_tensor_tensor(
                        xt[c][:],
                        ab[:],
                        s1,
                        bm[c][:],
                        Alu.subtract,
                        Alu.mult,
                        accum_out=col(acc2, c * 2 + 0),
                    )
                    g2 = b2_pool.tile([P, CW], BF16, name=f"g2_{b}_{c}", tag="b2")
                    nc.scalar.activation(
                        g2[:], xt[c][:], Act.Square, accum_out=col(acc2, c * 2 + 1)
                    )

                pair_reduce(st2[:], acc2[:], 2, red2)
                sq, sq2 = col(st2, 0), col(st2, 1)
                mean2, nm2, e2 = col(sv, 9), col(sv, 10), col(sv, 11)
                nv2, v2c, s2, kk = col(sv, 12), col(sv, 13), col(sv, 14), col(sv, 15)

                nc.vector.tensor_mul(mean2, sq, inv)
                nc.vector.tensor_scalar(nm2, mean2, -1.0, None, Alu.mult)
                nc.vector.tensor_mul(e2, sq2, inv)
                nc.vector.tensor_mul(nv2, mean2, mean2)
                nc.vector.tensor_sub(tE, e2, nv2)
                nc.vector.tensor_scalar(v2c, tE, C2, 1e-30, Alu.mult, Alu.max)
                nc.scalar.activation(tC, v2c, Act.Sqrt)
                newton_sqrt(s2, tC, v2c, tA, tB, tD, tF)
                nc.vector.tensor_add(kk, mean1, mean2)

                # ------------- stage 3: output assembly ----------------------
                for c in range(NCH):
                    b2t = b2_pool.tile([P, CW], BF16, name=f"b2_{b}_{c}", tag="b2")
                    nc.scalar.activation(b2t[:], xt[c][:], Act.Sign, bias=nm2)
                    p1 = w_pool.tile([P, CW], F32, name=f"p1_{b}_{c}", tag="w")
                    # p1 = s1*b1m + K  (TS dual-scalar, 2x)
                    nc.vector.tensor_scalar(p1[:], bm[c][:], s1, kk, Alu.mult, Alu.add)
                    p2 = w_pool.tile([P, CW], F32, name=f"p2_{b}_{c}", tag="w")
                    # p2 = s2*b2 + p1
                    nc.vector.scalar_tensor_tensor(
                        p2[:], b2t[:], s2, p1[:], Alu.mult, Alu.add
                    )
                    oc = w_pool.tile([P, CW], F32, name=f"oc_{b}_{c}", tag="w")
                    nc.vector.tensor_mul(oc[:], p2[:], m16[c][:])
                    nc.sync.dma_start(
                        out[r0 : r0 + P, c * CW : (c + 1) * CW], oc[:]
                    )

    return nc


def get_program():
    if "nc" not in _CACHE:
        nc = _build_program()
        # Bacc defers register allocation etc. to compile()/finalize();
        # the spmd exec path serializes without finalizing.
        nc.finalize()
        _CACHE["nc"] = nc
    return _CACHE["nc"]


def kernel(x: np.ndarray, mask: np.ndarray) -> np.ndarray:
    from concourse.bass_utils import run_bass_kernel_spmd

    x = np.ascontiguousarray(np.asarray(x, dtype=np.float32))
    mask_u8 = np.ascontiguousarray(np.asarray(mask)).view(np.uint8)
    assert x.shape == (R * NCORES, N), x.shape
    assert mask_u8.shape == (R * NCORES, N), mask_u8.shape

    nc = get_program()
    in_maps = [
        {
            "x": x[k * R : (k + 1) * R],
            "mask": mask_u8[k * R : (k + 1) * R],
        }
        for k in range(NCORES)
    ]
    res = run_bass_kernel_spmd(nc, in_maps, core_ids=list(range(NCORES)))
    return np.concatenate([r["out"] for r in res.results], axis=0)


if __name__ == "__main__":
    xs = np.random.randn(R * NCORES, N).astype(np.float32)
    ms = (np.random.randint(0, 2, (R * NCORES, N))).astype(bool)
    y = kernel(xs, ms)
    print(y.shape, y.dtype)


# revision 26
# speedup vs baseline: 1.0636x; 1.0309x over previous
"""Trainium2 Bass kernel for adaptive_high_order_residual_v2 (ORDER=2 masked
sign-binarization, per-row stats).

Full-input contract: kernel(x, mask) takes the complete (4096, 11008) arrays,
shards rows across 8 NeuronCores (512 rows each; per-row reductions make this
embarrassingly parallel), runs one SPMD Bass program, and concatenates the
per-core outputs.

Math per row (ORDER = 2, exact restructuring of the reference):
  t    = x * m                      (masked input)
  mean1 = sum(t)/cnt ; var1 = sum(t^2)/cnt - mean1^2 ; s1 = sqrt(var1 * 2/pi)
  b1   = sign(x - mean1)            (valid entries only; invalid masked later)
  q    = (|x - mean1| - s1) * b1*m  (== residual_2 of the reference: d - s1*b1)
  mean2 = sum(q)/cnt ; var2 = sum(q^2)/cnt - mean2^2 ; s2 = sqrt(var2 * 2/pi)
  b2   = sign(q - mean2)
  out  = ((mean1 + mean2) + s1*b1 + s2*b2) * m

Engine split per 128x2752 chunk:
  ACT: mask cast (+cnt accum), square(T) (+r2), Sign->b1, Abs->|d|,
       square(q) (+sum q^2), Sign->b2
  DVE: T=x*m (+r1 accum), b1m=b1*m16 (bf16 2x), q=(|d|-s1)*b1m (+sum q),
       p1=s1*b1m+K (2x), p2=s2*b2+p1, out=p2*m
"""

import sys

import numpy as np

sys.path.insert(0, "/opt/trn_rl_repo")

R = 512          # rows per core
N = 11008        # columns
P = 128          # SBUF partitions per row-block
NBLK = R // P    # 4 blocks per core
CW = 2752        # column chunk width
NCH = N // CW    # 4 chunks per block
NCORES = 8
C2 = 0.6366197723675814  # 2/pi

_CACHE = {}


def _build_program():
    import concourse.bacc as bacc
    import concourse.mybir as mybir
    from concourse.tile import TileContext

    F32 = mybir.dt.float32
    BF16 = mybir.dt.bfloat16
    U8 = mybir.dt.uint8
    Alu = mybir.AluOpType
    Act = mybir.ActivationFunctionType

    nc = bacc.Bacc()
    x = nc.dram_tensor("x", [R, N], F32, kind="ExternalInput")
    mk = nc.dram_tensor("mask", [R, N], U8, kind="ExternalInput")
    out = nc.dram_tensor("out", [R, N], F32, kind="ExternalOutput")

    with TileContext(nc) as tc:
        with (
            tc.tile_pool(name="xq", bufs=6) as xq_pool,    # x tile, later holds q
            tc.tile_pool(name="m8", bufs=3) as m8_pool,    # raw u8 mask (cast only)
            tc.tile_pool(name="m16", bufs=5) as m16_pool,  # bf16 mask
            tc.tile_pool(name="b1p", bufs=3) as b1_pool,   # unmasked sign1
            tc.tile_pool(name="bmp", bufs=5) as bm_pool,   # masked sign1 (bf16)
            tc.tile_pool(name="b2p", bufs=3) as b2_pool,   # sign2 + ACT garbage
            tc.tile_pool(name="w", bufs=4) as w_pool,      # f32 rotating work
            tc.tile_pool(name="sc", bufs=2) as sc_pool,    # scalars + accums
        ):
            for b in range(NBLK):
                r0 = b * P

                xt = [
                    xq_pool.tile([P, CW], F32, name=f"xt{b}_{c}", tag="xq")
                    for c in range(NCH)
                ]
                mt = [
                    m8_pool.tile([P, CW], U8, name=f"mt{b}_{c}", tag="m8")
                    for c in range(NCH)
                ]
                m16 = [
                    m16_pool.tile([P, CW], BF16, name=f"m16_{b}_{c}", tag="m16")
                    for c in range(NCH)
                ]
                bm = [
                    bm_pool.tile([P, CW], BF16, name=f"bm{b}_{c}", tag="bm")
                    for c in range(NCH)
                ]
                # accumulators, chunk-major interleave: col = c*nq + q so the
                # pairwise tree reduce uses contiguous 2-D slices
                acc1 = sc_pool.tile([P, 3 * NCH], F32, name=f"acc1_{b}", tag="acc1")
                acc2 = sc_pool.tile([P, 2 * NCH], F32, name=f"acc2_{b}", tag="acc2")
                st1 = sc_pool.tile([P, 3], F32, name=f"st1_{b}", tag="st1")
                st2 = sc_pool.tile([P, 2], F32, name=f"st2_{b}", tag="st2")
                red1 = sc_pool.tile([P, 6], F32, name=f"red1_{b}", tag="red1")
                red2 = sc_pool.tile([P, 4], F32, name=f"red2_{b}", tag="red2")
                sv = sc_pool.tile([P, 24], F32, name=f"sv_{b}", tag="sv")

                def col(t, i):
                    return t[:, i : i + 1]

                # ------------- stage 1: masked first-order stats -------------
                for c in range(NCH):
                    nc.sync.dma_start(xt[c][:], x[r0 : r0 + P, c * CW : (c + 1) * CW])
                    nc.sync.dma_start(mt[c][:], mk[r0 : r0 + P, c * CW : (c + 1) * CW])
                    # mask cast to bf16 + cnt partial
                    nc.scalar.activation(
                        m16[c][:], mt[c][:], Act.Copy, accum_out=col(acc1, c * 3 + 0)
                    )
                    # T = x*m + r1 partial
                    tt = w_pool.tile([P, CW], F32, name=f"tt{b}_{c}", tag="w")
                    nc.vector.scalar_tensor_tensor(
                        tt[:],
                        xt[c][:],
                        1.0,
                        mt[c][:],
                        Alu.bypass,
                        Alu.mult,
                        accum_out=col(acc1, c * 3 + 1),
                    )
                    # r2 partial: sum(T^2); output value unused
                    g1 = b2_pool.tile([P, CW], BF16, name=f"g1_{b}_{c}", tag="b2")
                    nc.scalar.activation(
                        g1[:], tt[:], Act.Square, accum_out=col(acc1, c * 3 + 2)
                    )

                # pairwise chunk reduce ([P, nq*NCH] -> [P, nq]); plain TT adds
                def pair_reduce(dst, acc, nq, red):
                    h = 2 * nq
                    nc.vector.tensor_add(red[:, 0:h], acc[:, 0:h], acc[:, h : 2 * h])
                    nc.vector.tensor_add(dst, red[:, 0:nq], red[:, nq:h])

                pair_reduce(st1[:], acc1[:], 3, red1)
                cnt, r1, r2 = col(st1, 0), col(st1, 1), col(st1, 2)
                cntc, inv = col(sv, 0), col(sv, 1)
                mean1, nm1, e1 = col(sv, 2), col(sv, 3), col(sv, 4)
                nv1, v1c, s1 = col(sv, 5), col(sv, 6), col(sv, 7)
                tA, tB, tC, tD = col(sv, 16), col(sv, 17), col(sv, 18), col(sv, 19)
                tE, tF = col(sv, 20), col(sv, 21)

                def newton_sqrt(dst, seed, vsq, t1, t2, mid, vh):
                    # dst = sqrt(vsq), one Newton step from the ACT seed (HW
                    # Sqrt is ~7e-6 rel; one step lands ~2e-11).
                    # TT/TS only (the STT ISA struct allows one sync wait).
                    nc.vector.tensor_scalar(vh[:], vsq[:], 0.5, None, Alu.mult)
                    nc.vector.reciprocal(t1[:], seed[:])
                    nc.vector.tensor_mul(t2[:], vh[:], t1[:])
                    nc.vector.tensor_scalar(t1[:], seed[:], 0.5, None, Alu.mult)
                    nc.vector.tensor_add(dst, t1[:], t2[:])

                nc.vector.tensor_scalar(cntc, cnt, 1.0, None, Alu.max)
                nc.vector.reciprocal(inv, cntc)
                nc.vector.tensor_mul(mean1, r1, inv)
                nc.vector.tensor_scalar(nm1, mean1, -1.0, None, Alu.mult)
                nc.vector.tensor_mul(e1, r2, inv)
                nc.vector.tensor_mul(nv1, mean1, mean1)
                nc.vector.tensor_sub(tE, e1, nv1)
                nc.vector.tensor_scalar(v1c, tE, C2, 1e-30, Alu.mult, Alu.max)
                nc.scalar.activation(tC, v1c, Act.Sqrt)
                newton_sqrt(s1, tC, v1c, tA, tB, tD, tF)

                # ------------- stage 2: residual q + second-order stats ------
                for c in range(NCH):
                    b1t = b1_pool.tile([P, CW], BF16, name=f"b1_{b}_{c}", tag="b1")
                    nc.scalar.activation(b1t[:], xt[c][:], Act.Sign, bias=nm1)
                    ab = w_pool.tile([P, CW], F32, name=f"ab{b}_{c}", tag="w")
                    nc.scalar.activation(ab[:], xt[c][:], Act.Abs, bias=nm1)
                    # masked sign1 (bf16 2x)
                    nc.vector.tensor_mul(bm[c][:], b1t[:], m16[c][:])
                    # q = (|d| - s1) * b1m, overwrites the x tile; accum sum(q)
                    nc.vector.scalar_tensor_tensor(
                        xt[c][:],
                        ab[:],
                        s1,
                        bm[c][:],
                        Alu.subtract,
                        Alu.mult,
                        accum_out=col(acc2, c * 2 + 0),
                    )
                    g2 = b2_pool.tile([P, CW], BF16, name=f"g2_{b}_{c}", tag="b2")
                    nc.scalar.activation(
                        g2[:], xt[c][:], Act.Square, accum_out=col(acc2, c * 2 + 1)
                    )

                pair_reduce(st2[:], acc2[:], 2, red2)
                sq, sq2 = col(st2, 0), col(st2, 1)
                mean2, nm2, e2 = col(sv, 9), col(sv, 10), col(sv, 11)
                nv2, v2c, s2, kk = col(sv, 12), col(sv, 13), col(sv, 14), col(sv, 15)

                nc.vector.tensor_mul(mean2, sq, inv)
                nc.vector.tensor_scalar(nm2, mean2, -1.0, None, Alu.mult)
                nc.vector.tensor_mul(e2, sq2, inv)
                nc.vector.tensor_mul(nv2, mean2, mean2)
                nc.vector.tensor_sub(tE, e2, nv2)
                nc.vector.tensor_scalar(v2c, tE, C2, 1e-30, Alu.mult, Alu.max)
                nc.scalar.activation(tC, v2c, Act.Sqrt)
                newton_sqrt(s2, tC, v2c, tA, tB, tD, tF)
                nc.vector.tensor_add(kk, mean1, mean2)

                # ------------- stage 3: output assembly ----------------------
                for c in range(NCH):
                    b2t = b2_pool.tile([P, CW], BF16, name=f"b2_{b}_{c}", tag="b2")
                    nc.scalar.activation(b2t[:], xt[c][:], Act.Sign, bias=nm2)
                    p1 = w_pool.tile([P, CW], F32, name=f"p1_{b}_{c}", tag="w")
                    # p1 = s1*b1m + K  (TS dual-scalar, 2x)
                    nc.vector.tensor_scalar(p1[:], bm[c][:], s1, kk, Alu.mult, Alu.add)
                    # p1 += s2*b2, then *= m  (in-place, one work tile/chunk)
                    nc.vector.scalar_tensor_tensor(
                        p1[:], b2t[:], s2, p1[:], Alu.mult, Alu.add
                    )
                    nc.vector.tensor_mul(p1[:], p1[:], m16[c][:])
                    nc.sync.dma_start(
                        out[r0 : r0 + P, c * CW : (c + 1) * CW], p1[:]
                    )

    return nc


def get_program():
    if "nc" not in _CACHE:
        nc = _build_program()
        # Bacc defers register allocation etc. to compile()/finalize();
        # the spmd exec path serializes without finalizing.
        nc.finalize()
        _CACHE["nc"] = nc
    return _CACHE["nc"]


def kernel(x: np.ndarray, mask: np.ndarray) -> np.ndarray:
    from concourse.bass_utils import run_bass_kernel_spmd

    x = np.ascontiguousarray(np.asarray(x, dtype=np.float32))
    mask_u8 = np.ascontiguousarray(np.asarray(mask)).view(np.uint8)
    assert x.shape == (R * NCORES, N), x.shape
    assert mask_u8.shape == (R * NCORES, N), mask_u8.shape

    nc = get_program()
    in_maps = [
        {
            "x": x[k * R : (k + 1) * R],
            "mask": mask_u8[k * R : (k + 1) * R],
        }
        for k in range(NCORES)
    ]
    res = run_bass_kernel_spmd(nc, in_maps, core_ids=list(range(NCORES)))
    return np.concatenate([r["out"] for r in res.results], axis=0)


if __name__ == "__main__":
    xs = np.random.randn(R * NCORES, N).astype(np.float32)
    ms = (np.random.randint(0, 2, (R * NCORES, N))).astype(bool)
    y = kernel(xs, ms)
    print(y.shape, y.dtype)
